# revision 17
# baseline (speedup 1.0000x reference)
"""Conv2d 3x3 (stride 1, pad 1) Trainium2 Bass kernel.

Problem: x (32, 128, 56, 56) fp32, kernels (256, 128, 3, 3) fp32, b (256,) fp32
-> out (32, 256, 56, 56) fp32.

Strategy:
  - Data-parallel over batch: 32 images / 8 cores = 4 images per core. SPMD,
    no collectives.
  - Per core: contraction dim C_in=128 lives on SBUF partitions. The 3x3 conv
    is 9 shifted [128c_in x 128c_out x <=448] matmuls accumulated in PSUM (one
    per kernel tap). Zero padding is implicit: boundary taps write a ragged
    sub-window of the PSUM tile and skip the rows/cols a zero pad would have
    contributed to; the center tap goes first and writes the full window with
    start=True so every element is initialized.
  - Output tiled as [c_out half (128 partitions), 8 rows x 56 cols = 448 free]
    (one PSUM bank). 2 halves x 7 row blocks x 4 images = 56 accumulation
    groups of 9 matmuls each per core.
  - Inputs bf16 (2x fp32 PE rate; fp32 PSUM accumulation). Bias added during
    PSUM->SBUF eviction on ScalarE; eviction writes bf16, host upcasts.
  - The matmul stream runs at the N/2.4GHz issue roofline (LDWEIGHTS hidden
    by the PE reorder window), so the wins are all in choreography:
      * The HAM clock gate needs one FULL free-running 3.4us window of
        uninterrupted PE-busy to lift 1.2->2.4 GHz, and any PE gap before the
        flip re-arms it.  Warm-up matmuls start right after the preamble and
        the first four real groups (blocks 0-1 x both halves) are emitted
        TAP-PHASED: 4 matmuls of tap 0, then 4x taps 1-2, 4x taps 3-5,
        4x taps 6-8 — so the cold-rate consumption never outruns the DMA
        supply and the PE never gaps.
      * DMA queues: completion of a queue's first transfer is delayed by
        every transfer kicked behind it (descriptors interleave), so each
        queue gets exactly one startup-critical transfer: scalar = tap 0
        (64KB, gates the first matmul), sync = chunk0, gpsimd (slow SW
        queue) = taps 3-5 (not needed until ~3us in). taps 1-2 follow on
        scalar, chunk1 + taps 6-8 follow on sync.
      * Last row block split [6,2] so the final PSUM->SBUF->HBM drain is
        ~28KB, kicked on the sync queue whose ring is hot from the store
        stream.
"""

import numpy as np
import ml_dtypes

import concourse.bass as bass
import concourse.tile as tile
from concourse import bacc, mybir
from concourse.bass_utils import run_bass_kernel_spmd

N_CORES = 8
N_FULL = 32
N_PER = N_FULL // N_CORES  # 4 images per core
C_IN = 128
C_OUT = 256
H = W = 56
HW = H * W
KS = 3
R = 8              # output rows per matmul group
NB = H // R        # 7 row blocks
NFREE = R * W      # 448 <= 512 (one PSUM bank of fp32)

_DT = mybir.dt.bfloat16

# Tap order: center tap (kh=1, kw=1) first — it writes the FULL window, so
# start=True initializes every PSUM element and the ragged boundary taps can
# accumulate into sub-windows.
TAPS = [(1, 1)] + [
    (kh, kw) for kh in range(KS) for kw in range(KS) if not (kh == 1 and kw == 1)
]
# Weight DMA parts (tap ranges, both c_out halves each): taps 0-2 are one
# 192KB transfer on scalar (the first-matmul gate — one merged transfer
# completes much earlier than a chain of small ones, whose completions are
# delayed by every transfer queued behind them); taps 3-5 on the gpsimd SW
# queue; taps 6-8 behind chunk0 on sync.
WPARTS = [(0, 3), (3, 6), (6, 9)]

# Warm-up: matmuls of N=128 (~107ns each cold) on a small zeroed tile keep
# the PE busy from the end of the preamble (~7.1us) until the tap-0-2
# weights land (~10.2us), so the HAM busy window starts as early as possible
# and never gaps before the 1.2->2.4 GHz flip.
N_WARM = 30
WARM_N = 128

# Final row block split: the 2-row tail group makes the last eviction+store
# tiny (the end-of-kernel barrier waits on its DMA flight).
LAST_SPLIT = [6, 2]


def _build():
    nc = bacc.Bacc(
        "TRN2",
        target_bir_lowering=False,
        debug=False,
        num_devices=N_CORES,
    )
    xs = nc.dram_tensor("xs", [N_PER, C_IN, H, W], _DT, kind="ExternalInput").ap()
    wt = nc.dram_tensor("wt", [C_IN, KS * KS * C_OUT], _DT, kind="ExternalInput").ap()
    bt = nc.dram_tensor("bt", [128, 2], mybir.dt.float32, kind="ExternalInput").ap()
    y = nc.dram_tensor("y", [N_PER, C_OUT, HW], _DT, kind="ExternalOutput").ap()

    with tile.TileContext(nc) as tc:
        with (
            tc.tile_pool(name="const", bufs=1) as const,
            tc.tile_pool(name="wpool", bufs=1, space="PSUM") as wpool,
            tc.tile_pool(name="xpool", bufs=N_PER * NB) as xpool,
            tc.tile_pool(name="pspool", bufs=7, space="PSUM") as pspool,
            tc.tile_pool(name="opool", bufs=4) as opool,
        ):
            # Warm-up matmuls depend only on a small vector memset (vector
            # never issues DMAs, so the DMA-capable engines kick their first
            # transfers immediately after the preamble barrier).
            warm = const.tile([128, WARM_N], _DT)
            nc.vector.memset(warm[:], 0.0)
            wps = wpool.tile([128, 512], mybir.dt.float32)
            for i in range(N_WARM):
                nc.tensor.matmul(
                    wps[:, :WARM_N],
                    lhsT=warm[:, :128],
                    rhs=warm[:, :WARM_N],
                    start=(i == 0),
                    stop=(i == N_WARM - 1),
                )

            # Weight parts: scalar gets tap0 (the first-matmul gate) and then
            # taps 1-2; gpsimd (software queue, slow first transfer but kicked
            # immediately) gets taps 3-5 + bias; sync gets taps 6-8 after
            # chunk0+chunk1 (see below).
            wparts = []
            for p, (lo, hi) in enumerate(WPARTS):
                wp_sb = const.tile(
                    [C_IN, (hi - lo) * C_OUT], _DT, name=f"wt_sb{p}"
                )
                wparts.append(wp_sb)
            nc.scalar.dma_start(out=wparts[0][:], in_=wt[:, 0 : 3 * C_OUT])
            nc.gpsimd.dma_start(
                out=wparts[1][:], in_=wt[:, 3 * C_OUT : 6 * C_OUT]
            )
            bias_sb = const.tile([128, 2], mybir.dt.float32)
            nc.gpsimd.dma_start(out=bias_sb[:], in_=bt)

            def wt_ap(idx, half):
                p = next(i for i, (lo, hi) in enumerate(WPARTS) if idx < hi)
                off = (idx - WPARTS[p][0]) * C_OUT + half * 128
                return wparts[p][:, off : off + 128]

            def tap_windows(r0, nrows):
                wins = []
                for idx, (kh, kw) in enumerate(TAPS):
                    dh, dw = kh - 1, kw - 1
                    rlo = max(r0, -dh)
                    rhi = min(r0 + nrows, H - dh)
                    if rlo >= rhi:
                        continue
                    clo = max(0, -dw)
                    chi = min(W, W - dw)
                    wins.append((idx, dh, dw, rlo, rhi, clo, chi))
                return wins

            def emit_tap(ps3, xc, rb, r0, half, win, start, stop):
                idx, dh, dw, rlo, rhi, clo, chi = win
                nc.tensor.matmul(
                    ps3[:, rlo - r0 : rhi - r0, clo:chi],
                    lhsT=wt_ap(idx, half),
                    rhs=xc[
                        :,
                        rlo + dh - (rb * R) + 1 : rhi + dh - (rb * R) + 1,
                        clo + dw : chi + dw,
                    ],
                    start=start,
                    stop=stop,
                )

            def emit_evict(ps, half, n, r0, nfree, store_eng=None):
                ot = opool.tile([128, NFREE], _DT, tag="ot")
                nc.scalar.activation(
                    ot[:, :nfree],
                    ps[:, :nfree],
                    mybir.ActivationFunctionType.Identity,
                    bias=bias_sb[:, half : half + 1],
                    scale=1.0,
                )
                y_slice = y[
                    n, half * 128 : (half + 1) * 128, r0 * W : r0 * W + nfree
                ]
                (store_eng or nc.sync).dma_start(out=y_slice, in_=ot[:, :nfree])

            chunks_all = {}

            def load_chunk(n, c, eng=None):
                xc = xpool.tile([C_IN, R + 2, W], _DT, tag="xc", name=f"xc{n}_{c}")
                lo = max(0, c * R - 1)
                hi = min(H, c * R + R + 1)
                (eng or nc.sync).dma_start(
                    out=xc[:, lo - (c * R - 1) : hi - (c * R - 1), :],
                    in_=xs[n, :, lo:hi, :],
                )
                chunks_all[(n, c)] = xc

            # --- startup: image 0, blocks 0-1, both halves, tap-phased ---
            # chunk1 rides scalar's 2nd slot (behind taps 0-2, lands ~11.9us)
            # so block-0's taps 0-2 run first; taps 6-8 ride sync's 2nd slot
            # behind chunk0 (consumed ~4us after the first real matmul)
            load_chunk(0, 0)
            load_chunk(0, 1, eng=nc.scalar)
            nc.sync.dma_start(
                out=wparts[2][:], in_=wt[:, 6 * C_OUT : 9 * C_OUT]
            )
            first4 = []  # (ps, ps3, xc, rb, r0, half, wins)
            for rb in range(2):
                for half in range(2):
                    ps = pspool.tile([128, NFREE], mybir.dt.float32, tag="ps")
                    ps3 = ps[:, :NFREE].rearrange("p (r c) -> p r c", r=R)
                    first4.append(
                        (ps, ps3, chunks_all[(0, rb)], rb, rb * R, half,
                         tap_windows(rb * R, R))
                    )

            def emit_phase(groups, phase):
                for (ps, ps3, xc, rb, r0, half, wins) in groups:
                    for win in wins:
                        if win[0] not in phase:
                            continue
                        emit_tap(ps3, xc, rb, r0, half, win,
                                 start=(win[0] == 0), stop=(win[0] == 8))

            # taps 0-2 for block 0 (needs only chunk0 + the scalar weights),
            # then block 1 once chunk1 has landed, then taps 3-5 / 6-8 for
            # all four groups
            emit_phase(first4[:2], [0, 1, 2])
            emit_phase(first4[2:], [0, 1, 2])
            load_chunk(0, 2)
            load_chunk(0, 3)
            emit_phase(first4, [3, 4, 5])
            emit_phase(first4, [6, 7, 8])
            for (ps, ps3, xc, rb, r0, half, wins) in first4:
                emit_evict(ps, half, 0, r0, NFREE)

            # --- image 0, blocks 2-6, then the normal loop ---
            for n in range(N_PER):
                for half in range(2):
                    for rb in range(NB):
                        if n == 0 and rb < 2:
                            continue  # done in the startup phase
                        if half == 0 and rb + 2 < NB:
                            load_chunk(n, rb + 2)
                        xc = chunks_all[(n, rb)]
                        last_block = n == N_PER - 1 and half == 1 and rb == NB - 1
                        if last_block:
                            sub = []
                            off = 0
                            for nr in LAST_SPLIT:
                                sub.append((off, nr))
                                off += nr
                        else:
                            sub = [(0, R)]
                        for si, (soff, nrows) in enumerate(sub):
                            nfree = nrows * W
                            ps = pspool.tile([128, NFREE], mybir.dt.float32, tag="ps")
                            ps3 = ps[:, :nfree].rearrange(
                                "p (r c) -> p r c", r=nrows
                            )
                            r0 = rb * R + soff
                            wins = tap_windows(r0, nrows)
                            for wi, win in enumerate(wins):
                                emit_tap(ps3, xc, rb, r0, half, win,
                                         start=(wi == 0),
                                         stop=(wi == len(wins) - 1))
                            # final store goes on the sync queue (hot ring);
                            # the earlier sub-block store on scalar
                            eng = None
                            if last_block and si == 0:
                                eng = nc.scalar
                            emit_evict(ps, half, n, r0, nfree, store_eng=eng)
                # next image's first two chunks load during this image's
                # second half so they are ready at the image boundary
                if n + 1 < N_PER:
                    load_chunk(n + 1, 0)
                    load_chunk(n + 1, 1)
    nc.compile()
    return nc


_NC = None


def _get_nc():
    global _NC
    if _NC is None:
        _NC = _build()
    return _NC


def _prep_inputs(x, kernels, b):
    bf16 = ml_dtypes.bfloat16
    xb = np.ascontiguousarray(x, dtype=np.float32).astype(bf16)
    # [O, I, kh, kw] -> [I, tap, O] in TAPS order -> [128, 9*256]
    wk = np.transpose(np.asarray(kernels, dtype=np.float32), (1, 2, 3, 0))
    wtb = np.ascontiguousarray(
        np.stack([wk[:, kh, kw, :] for kh, kw in TAPS], axis=1)
    ).reshape(C_IN, KS * KS * C_OUT).astype(bf16)
    # bias [256] -> [128, 2]: column h holds b[h*128 : (h+1)*128]
    btb = np.ascontiguousarray(
        np.asarray(b, dtype=np.float32).reshape(2, 128).T
    )
    return xb, wtb, btb


def kernel(x, kernels, b):
    nc = _get_nc()
    xb, wtb, btb = _prep_inputs(x, kernels, b)
    in_maps = [
        {"xs": xb[i * N_PER : (i + 1) * N_PER], "wt": wtb, "bt": btb}
        for i in range(N_CORES)
    ]
    res = run_bass_kernel_spmd(nc, in_maps, core_ids=list(range(N_CORES)))
    out = np.concatenate(
        [
            np.asarray(r["y"], dtype=np.float32).reshape(N_PER, C_OUT, H, W)
            for r in res.results
        ],
        axis=0,
    )
    return np.ascontiguousarray(out, dtype=np.float32)


# revision 20
# speedup vs baseline: 1.0064x; 1.0064x over previous
"""Conv2d 3x3 (stride 1, pad 1) Trainium2 Bass kernel.

Problem: x (32, 128, 56, 56) fp32, kernels (256, 128, 3, 3) fp32, b (256,) fp32
-> out (32, 256, 56, 56) fp32.

Strategy:
  - Data-parallel over batch: 32 images / 8 cores = 4 images per core. SPMD,
    no collectives.
  - Per core: contraction dim C_in=128 lives on SBUF partitions. The 3x3 conv
    is 9 shifted [128c_in x 128c_out x <=448] matmuls accumulated in PSUM (one
    per kernel tap). Zero padding is implicit: boundary taps write a ragged
    sub-window of the PSUM tile and skip the rows/cols a zero pad would have
    contributed to; the center tap goes first and writes the full window with
    start=True so every element is initialized.
  - Output tiled as [c_out half (128 partitions), 8 rows x 56 cols = 448 free]
    (one PSUM bank). 2 halves x 7 row blocks x 4 images = 56 accumulation
    groups of 9 matmuls each per core.
  - Inputs bf16 (2x fp32 PE rate; fp32 PSUM accumulation). Bias added during
    PSUM->SBUF eviction on ScalarE; eviction writes bf16, host upcasts.
  - The matmul stream runs at the N/2.4GHz issue roofline (LDWEIGHTS hidden
    by the PE reorder window), so the wins are all in choreography:
      * The HAM clock gate needs one FULL free-running 3.4us window of
        uninterrupted PE-busy to lift 1.2->2.4 GHz, and any PE gap before the
        flip re-arms it.  Warm-up matmuls start right after the preamble and
        the first four real groups (blocks 0-1 x both halves) are emitted
        TAP-PHASED: 4 matmuls of tap 0, then 4x taps 1-2, 4x taps 3-5,
        4x taps 6-8 — so the cold-rate consumption never outruns the DMA
        supply and the PE never gaps.
      * DMA queues: completion of a queue's first transfer is delayed by
        every transfer kicked behind it (descriptors interleave), so each
        queue gets exactly one startup-critical transfer: scalar = tap 0
        (64KB, gates the first matmul), sync = chunk0, gpsimd (slow SW
        queue) = taps 3-5 (not needed until ~3us in). taps 1-2 follow on
        scalar, chunk1 + taps 6-8 follow on sync.
      * Last row block split [6,2] so the final PSUM->SBUF->HBM drain is
        ~28KB, kicked on the sync queue whose ring is hot from the store
        stream.
"""

import numpy as np
import ml_dtypes

import concourse.bass as bass
import concourse.tile as tile
from concourse import bacc, mybir
from concourse.bass_utils import run_bass_kernel_spmd

N_CORES = 8
N_FULL = 32
N_PER = N_FULL // N_CORES  # 4 images per core
C_IN = 128
C_OUT = 256
H = W = 56
HW = H * W
KS = 3
R = 8              # output rows per matmul group
NB = H // R        # 7 row blocks
NFREE = R * W      # 448 <= 512 (one PSUM bank of fp32)

_DT = mybir.dt.bfloat16

# Tap order: center tap (kh=1, kw=1) first — it writes the FULL window, so
# start=True initializes every PSUM element and the ragged boundary taps can
# accumulate into sub-windows.
TAPS = [(1, 1)] + [
    (kh, kw) for kh in range(KS) for kw in range(KS) if not (kh == 1 and kw == 1)
]
# Weight DMA parts (tap ranges, both c_out halves each): taps 0-2 are one
# 192KB transfer on scalar (the first-matmul gate — one merged transfer
# completes much earlier than a chain of small ones, whose completions are
# delayed by every transfer queued behind them); taps 3-5 on the gpsimd SW
# queue; taps 6-8 behind chunk0 on sync.
WPARTS = [(0, 3), (3, 6), (6, 9)]

# Warm-up: matmuls of N=128 (~107ns each cold) on a small zeroed tile keep
# the PE busy from the end of the preamble (~7.1us) until the tap-0-2
# weights land (~10.2us), so the HAM busy window starts as early as possible
# and never gaps before the 1.2->2.4 GHz flip.
N_WARM = 28
WARM_N = 128

# Final row block split: the 2-row tail group makes the last eviction+store
# tiny (the end-of-kernel barrier waits on its DMA flight).
LAST_SPLIT = [6, 2]


def _build():
    nc = bacc.Bacc(
        "TRN2",
        target_bir_lowering=False,
        debug=False,
        num_devices=N_CORES,
    )
    xs = nc.dram_tensor("xs", [N_PER, C_IN, H, W], _DT, kind="ExternalInput").ap()
    wt = nc.dram_tensor("wt", [C_IN, KS * KS * C_OUT], _DT, kind="ExternalInput").ap()
    bt = nc.dram_tensor("bt", [128, 2], mybir.dt.float32, kind="ExternalInput").ap()
    y = nc.dram_tensor("y", [N_PER, C_OUT, HW], _DT, kind="ExternalOutput").ap()

    with tile.TileContext(nc) as tc:
        with (
            tc.tile_pool(name="const", bufs=1) as const,
            tc.tile_pool(name="wpool", bufs=1, space="PSUM") as wpool,
            tc.tile_pool(name="xpool", bufs=N_PER * NB) as xpool,
            tc.tile_pool(name="pspool", bufs=7, space="PSUM") as pspool,
            tc.tile_pool(name="opool", bufs=4) as opool,
        ):
            # Warm-up matmuls depend only on a small vector memset (vector
            # never issues DMAs, so the DMA-capable engines kick their first
            # transfers immediately after the preamble barrier).
            warm = const.tile([128, WARM_N], _DT)
            nc.vector.memset(warm[:], 0.0)
            wps = wpool.tile([128, 512], mybir.dt.float32)
            for i in range(N_WARM):
                nc.tensor.matmul(
                    wps[:, :WARM_N],
                    lhsT=warm[:, :128],
                    rhs=warm[:, :WARM_N],
                    start=(i == 0),
                    stop=(i == N_WARM - 1),
                )

            # Weight parts: scalar gets tap0 (the first-matmul gate) and then
            # taps 1-2; gpsimd (software queue, slow first transfer but kicked
            # immediately) gets taps 3-5 + bias; sync gets taps 6-8 after
            # chunk0+chunk1 (see below).
            wparts = []
            for p, (lo, hi) in enumerate(WPARTS):
                wp_sb = const.tile(
                    [C_IN, (hi - lo) * C_OUT], _DT, name=f"wt_sb{p}"
                )
                wparts.append(wp_sb)
            nc.scalar.dma_start(out=wparts[0][:], in_=wt[:, 0 : 3 * C_OUT])
            nc.gpsimd.dma_start(
                out=wparts[1][:], in_=wt[:, 3 * C_OUT : 6 * C_OUT]
            )
            bias_sb = const.tile([128, 2], mybir.dt.float32)

            def wt_ap(idx, half):
                p = next(i for i, (lo, hi) in enumerate(WPARTS) if idx < hi)
                off = (idx - WPARTS[p][0]) * C_OUT + half * 128
                return wparts[p][:, off : off + 128]

            def tap_windows(r0, nrows):
                wins = []
                for idx, (kh, kw) in enumerate(TAPS):
                    dh, dw = kh - 1, kw - 1
                    rlo = max(r0, -dh)
                    rhi = min(r0 + nrows, H - dh)
                    if rlo >= rhi:
                        continue
                    clo = max(0, -dw)
                    chi = min(W, W - dw)
                    wins.append((idx, dh, dw, rlo, rhi, clo, chi))
                return wins

            def emit_tap(ps3, xc, rb, r0, half, win, start, stop):
                idx, dh, dw, rlo, rhi, clo, chi = win
                nc.tensor.matmul(
                    ps3[:, rlo - r0 : rhi - r0, clo:chi],
                    lhsT=wt_ap(idx, half),
                    rhs=xc[
                        :,
                        rlo + dh - (rb * R) + 1 : rhi + dh - (rb * R) + 1,
                        clo + dw : chi + dw,
                    ],
                    start=start,
                    stop=stop,
                )

            def emit_evict(ps, half, n, r0, nfree, store_eng=None):
                ot = opool.tile([128, NFREE], _DT, tag="ot")
                nc.scalar.activation(
                    ot[:, :nfree],
                    ps[:, :nfree],
                    mybir.ActivationFunctionType.Identity,
                    bias=bias_sb[:, half : half + 1],
                    scale=1.0,
                )
                y_slice = y[
                    n, half * 128 : (half + 1) * 128, r0 * W : r0 * W + nfree
                ]
                (store_eng or nc.sync).dma_start(out=y_slice, in_=ot[:, :nfree])

            chunks_all = {}

            def load_chunk(n, c, eng=None):
                xc = xpool.tile([C_IN, R + 2, W], _DT, tag="xc", name=f"xc{n}_{c}")
                lo = max(0, c * R - 1)
                hi = min(H, c * R + R + 1)
                (eng or nc.sync).dma_start(
                    out=xc[:, lo - (c * R - 1) : hi - (c * R - 1), :],
                    in_=xs[n, :, lo:hi, :],
                )
                chunks_all[(n, c)] = xc

            # --- startup: image 0, blocks 0-1, both halves, tap-phased ---
            # the scalar queue carries ONLY taps 0-2 (the first-matmul gate:
            # anything kicked behind it delays its completion); chunk1 + bias
            # ride gpsimd's 2nd/3rd slots, taps 6-8 ride sync's 2nd slot
            # behind chunk0 (consumed ~4us after the first real matmul)
            load_chunk(0, 0)
            load_chunk(0, 1, eng=nc.gpsimd)
            nc.gpsimd.dma_start(out=bias_sb[:], in_=bt)
            nc.sync.dma_start(
                out=wparts[2][:], in_=wt[:, 6 * C_OUT : 9 * C_OUT]
            )
            first4 = []  # (ps, ps3, xc, rb, r0, half, wins)
            for rb in range(2):
                for half in range(2):
                    ps = pspool.tile([128, NFREE], mybir.dt.float32, tag="ps")
                    ps3 = ps[:, :NFREE].rearrange("p (r c) -> p r c", r=R)
                    first4.append(
                        (ps, ps3, chunks_all[(0, rb)], rb, rb * R, half,
                         tap_windows(rb * R, R))
                    )

            def emit_phase(groups, phase):
                for (ps, ps3, xc, rb, r0, half, wins) in groups:
                    for win in wins:
                        if win[0] not in phase:
                            continue
                        emit_tap(ps3, xc, rb, r0, half, win,
                                 start=(win[0] == 0), stop=(win[0] == 8))

            # taps 0-2 for block 0 (needs only chunk0 + the scalar weights),
            # then block 1 once chunk1 has landed, then taps 3-5 / 6-8 for
            # all four groups
            emit_phase(first4[:2], [0, 1, 2])
            emit_phase(first4[2:], [0, 1, 2])
            load_chunk(0, 2)
            load_chunk(0, 3)
            emit_phase(first4, [3, 4, 5])
            emit_phase(first4, [6, 7, 8])
            for (ps, ps3, xc, rb, r0, half, wins) in first4:
                emit_evict(ps, half, 0, r0, NFREE)

            # --- image 0, blocks 2-6, then the normal loop ---
            for n in range(N_PER):
                for half in range(2):
                    for rb in range(NB):
                        if n == 0 and rb < 2:
                            continue  # done in the startup phase
                        if half == 0 and rb + 2 < NB:
                            load_chunk(n, rb + 2)
                        xc = chunks_all[(n, rb)]
                        last_block = n == N_PER - 1 and half == 1 and rb == NB - 1
                        if last_block:
                            sub = []
                            off = 0
                            for nr in LAST_SPLIT:
                                sub.append((off, nr))
                                off += nr
                        else:
                            sub = [(0, R)]
                        for si, (soff, nrows) in enumerate(sub):
                            nfree = nrows * W
                            ps = pspool.tile([128, NFREE], mybir.dt.float32, tag="ps")
                            ps3 = ps[:, :nfree].rearrange(
                                "p (r c) -> p r c", r=nrows
                            )
                            r0 = rb * R + soff
                            wins = tap_windows(r0, nrows)
                            for wi, win in enumerate(wins):
                                emit_tap(ps3, xc, rb, r0, half, win,
                                         start=(wi == 0),
                                         stop=(wi == len(wins) - 1))
                            # final store goes on the sync queue (hot ring);
                            # the earlier sub-block store on scalar
                            eng = None
                            if last_block and si == 0:
                                eng = nc.scalar
                            emit_evict(ps, half, n, r0, nfree, store_eng=eng)
                # next image's first two chunks load during this image's
                # second half so they are ready at the image boundary
                if n + 1 < N_PER:
                    load_chunk(n + 1, 0)
                    load_chunk(n + 1, 1)
    nc.compile()
    return nc


_NC = None


def _get_nc():
    global _NC
    if _NC is None:
        _NC = _build()
    return _NC


def _prep_inputs(x, kernels, b):
    bf16 = ml_dtypes.bfloat16
    xb = np.ascontiguousarray(x, dtype=np.float32).astype(bf16)
    # [O, I, kh, kw] -> [I, tap, O] in TAPS order -> [128, 9*256]
    wk = np.transpose(np.asarray(kernels, dtype=np.float32), (1, 2, 3, 0))
    wtb = np.ascontiguousarray(
        np.stack([wk[:, kh, kw, :] for kh, kw in TAPS], axis=1)
    ).reshape(C_IN, KS * KS * C_OUT).astype(bf16)
    # bias [256] -> [128, 2]: column h holds b[h*128 : (h+1)*128]
    btb = np.ascontiguousarray(
        np.asarray(b, dtype=np.float32).reshape(2, 128).T
    )
    return xb, wtb, btb


def kernel(x, kernels, b):
    nc = _get_nc()
    xb, wtb, btb = _prep_inputs(x, kernels, b)
    in_maps = [
        {"xs": xb[i * N_PER : (i + 1) * N_PER], "wt": wtb, "bt": btb}
        for i in range(N_CORES)
    ]
    res = run_bass_kernel_spmd(nc, in_maps, core_ids=list(range(N_CORES)))
    out = np.concatenate(
        [
            np.asarray(r["y"], dtype=np.float32).reshape(N_PER, C_OUT, H, W)
            for r in res.results
        ],
        axis=0,
    )
    return np.ascontiguousarray(out, dtype=np.float32)
